# revision 17
# baseline (speedup 1.0000x reference)
"""Trainium2 Bass kernel for nn_AttnHead (GAT-style attention head).

Reference computation per batch b:
    V   = seq @ W_fts                         [N, D]
    f1  = seq @ w_f1 + b_f1                   [N]
    f2  = seq @ w_f2 + b_f2                   [N]
    out = relu(softmax_m(lrelu(f1[n]+f2[m])) @ V + bias)

The logits are rank-1 (f1[n] + f2[m]) and exp(lrelu(x)) factorizes on each
side of x=0:
    exp(lrelu(f1+f2)) = e1[n]*e2[m]     if f1+f2 >= 0   (e = exp(f))
                      = e1s[n]*e2s[m]   otherwise       (es = exp(0.01 f))
so row n of the attention numerator is
    e1[n]*A(t_n) + e1s[n]*B(t_n),   t_n = -f1[n]
    A(t) = sum_{m: f2[m]>=t} e2[m] V[m,:]      (suffix sum over f2 order)
    B(t) = sum_{m: f2[m]<t}  e2s[m] V[m,:]     (prefix sum)
We evaluate A and the complementary suffix S2 on a uniform 128-bucket grid
over [min f2, max f2] via a smooth-staircase matmul (O(N*K*(2D+2)) instead
of O(N^2*D)), then linearly interpolate at each t_n with hat weights:
    out_unnorm[n] = sum_k hat_k(t_n) (e1[n] T1[k] - e1s[n] T2[k])
                    + e1s[n] * tot2          (since sum_k hat_k = 1)
where T1[k] = sum_m S(b_m-k) e2[m] [1|V], T2[k] = sum_m S(b_m-k) e2s[m[1|V],
tot2 = T2[k=-2] (the all-ones staircase column). All three terms accumulate
in one PSUM group (hat*e1, -hat*e1s, and a K=1 rank-1 matmul e1s x tot2).
Boundary smoothing is invisible at output precision because exp(lrelu) is
continuous at 0 and softmax normalization cancels smooth weight bias
(validated: absmax error ~9e-4 of output scale vs the fp32 reference).

Sharding: pure data-parallel, one batch per NeuronCore (B=8, 8 cores).
"""

import numpy as np

import concourse.bacc as bacc
import concourse.mybir as mybir
import concourse.tile as tile
from concourse import bass_isa
from concourse import library_config
from concourse.bass_utils import run_bass_kernel_spmd

F32 = mybir.dt.float32
F16 = mybir.dt.float16
AF = mybir.ActivationFunctionType
ALU = mybir.AluOpType
AX = mybir.AxisListType

N, D = 4096, 256
NCH = N // 128            # 32 m/n chunks of 128
KC = 128                  # staircase columns (grid buckets)
KEFF = float(KC - 7)      # usable buckets: b in [2, KC-5]
SHARP = 6.0               # sigmoid sharpness (in bucket units)
DC = D + 2                # phase-A matmul width: V | f1 | f2
UC = D + 1                # table width: weight col | weighted V
VS = D + 2                # Vsb chunk stride: V | f1 | f2
V16S = D + 1              # V16 chunk stride: 1 | V

# consts layout ([128, 264] fp32)
C_NEGK = 0       # cols 0:128   row p: 2.0 - j   (staircase ramp offsets)
C_IDN = 128      # cols 128:256 identity matrix (PE transpose operand)
C_B1 = 257       # col: b_f1
C_B2 = 258       # col: b_f2
C_BIAS = 259     # col: bias
C_B1S = 260      # col: 0.01*b_f1
C_B2S = 261      # col: 0.01*b_f2
C_B12 = 262      # col: b_f1 + b_f2
C_HB1 = 264      # col: 3.0 - p   (hat left arm:  relu(t + (3-p)))
C_HB2 = 265      # col: p - 1.0   (hat right arm: relu(-t + (p-1)))


def _emit(tc, seq_d, ww_d, consts_d, out_d):
    nc = tc.nc
    bounce_d = nc.dram_tensor("bounce", [3, N], F32).ap()
    bounce16_d = nc.dram_tensor("bounce16", [N], F16).ap()
    nc.gpsimd.load_library(library_config.attn)

    with (
        tc.tile_pool(name="const", bufs=1) as cpool,
        tc.tile_pool(name="small", bufs=1) as sm,
    ):
        consts = cpool.tile([128, 266], F32)
        nc.sync.dma_start(consts[:], consts_d[:])
        ww32 = cpool.tile([128, 2 * DC], F32)
        for dc in range(2):
            nc.sync.dma_start(
                ww32[:, dc * DC:(dc + 1) * DC], ww_d[dc * 128:(dc + 1) * 128, :]
            )
        ww = cpool.tile([128, 2 * DC], F16)
        nc.vector.tensor_copy(ww[:], ww32[:])
        negk = consts[:, C_NEGK:C_NEGK + KC]
        iden = consts[:, C_IDN:C_IDN + 128]

        def ccol(c):
            return consts[:, c:c + 1]

        e1 = sm.tile([128, NCH], F32)
        e1s = sm.tile([128, NCH], F32)
        e1sn = sm.tile([128, NCH], F32)        # -e1s
        e1s16 = sm.tile([128, NCH], F16)
        bc = sm.tile([128, NCH], F32)          # bucket coords of f2 (no +2)
        tcn = sm.tile([128, NCH], F32)         # clamped thresholds (no +2)
        e2sb = sm.tile([128, NCH], F32)        # exp(f2+b2)
        e2ssb = sm.tile([128, NCH], F32)       # exp(0.01(f2+b2))
        T1sb = sm.tile([128, UC], F16)
        T2sb = sm.tile([128, UC], F16)
        t2row = sm.tile([1, UC], F16)          # totals row (col k=-2)
        e1srow = sm.tile([1, N], F16)

        with (
            tc.tile_pool(name="psT", bufs=1, space="PSUM") as psT,
            tc.tile_pool(name="V", bufs=1) as Vp,
            tc.tile_pool(name="bcast", bufs=1) as bp,
            tc.tile_pool(name="q", bufs=1) as qp,
        ):
            T1raw = psT.tile([128, UC], F32, tag="t1")
            T2raw = psT.tile([128, UC], F32, tag="t2")
            Vsb = Vp.tile([128, NCH * VS], F32)     # [V | f1 | f2] per chunk
            V16 = Vp.tile([128, NCH * V16S], F16)   # [1 | V] per chunk
            f1v = Vsb[:].rearrange("p (c s) -> p s c", s=VS)[:, D, :]
            f2v = Vsb[:].rearrange("p (c s) -> p s c", s=VS)[:, D + 1, :]

            # ---------------- phase A: seqT then V,f1,f2 ----------------
            with (
                tc.tile_pool(name="seqraw", bufs=4) as rawp,
                tc.tile_pool(name="seqT", bufs=1) as seqTp,
                tc.tile_pool(name="psA", bufs=2, space="PSUM") as psA,
            ):
                seqT = seqTp.tile([128, 2 * N], F16)   # [d%128, dc*N + m]
                for c in range(NCH):
                    raw = rawp.tile([128, D], F32)
                    nc.sync.dma_start(raw[:], seq_d[c * 128:(c + 1) * 128, :])
                    ptr = psA.tile([128, 256], F32, tag="ptr")
                    for dc in range(2):
                        nc.tensor.transpose(
                            ptr[:, dc * 128:(dc + 1) * 128],
                            raw[:, dc * 128:(dc + 1) * 128],
                            iden,
                        )
                    dst = seqT[:].rearrange("p (dc m) -> p dc m", dc=2)[
                        :, :, c * 128:(c + 1) * 128
                    ]
                    src = ptr[:].rearrange("p (dc m) -> p dc m", dc=2)
                    if c % 2 == 0:
                        nc.vector.tensor_copy(dst, src)
                    else:
                        nc.scalar.copy(dst, src)

                for c in range(NCH):
                    pa = psA.tile([128, DC], F32, tag="pa")
                    for dc in range(2):
                        nc.tensor.matmul(
                            pa[:],
                            seqT[:, dc * N + c * 128: dc * N + (c + 1) * 128],
                            ww[:, dc * DC:(dc + 1) * DC],
                            start=(dc == 0),
                            stop=(dc == 1),
                        )
                    if c % 2 == 0:
                        nc.vector.tensor_copy(
                            Vsb[:, c * VS:(c + 1) * VS], pa[:]
                        )
                    else:
                        nc.scalar.copy(Vsb[:, c * VS:(c + 1) * VS], pa[:])

            # V16 = [1 | V] fp16: strided ones col + two half casts
            v16ones = V16[:].rearrange("p (c s) -> p s c", s=V16S)[:, 0:1, :]
            nc.vector.memset(v16ones, 1.0)
            half = NCH // 2
            src_v = Vsb[:].rearrange("p (c s) -> p c s", s=VS)[:, :, 0:D]
            dst_v = V16[:].rearrange("p (c s) -> p c s", s=V16S)[:, :, 1:]
            nc.vector.tensor_copy(dst_v[:, 0:half, :], src_v[:, 0:half, :])
            nc.scalar.copy(dst_v[:, half:, :], src_v[:, half:, :])

            # ---------------- grid + per-node scalars ----------------
            rmax = sm.tile([128, 1], F32)
            rminn = sm.tile([128, 1], F32)
            f2neg = sm.tile([128, NCH], F32)
            nc.vector.tensor_reduce(op=ALU.max, out=rmax[:], in_=f2v, axis=AX.X)
            nc.vector.tensor_scalar(f2neg[:], f2v, -1.0, None, ALU.mult)
            nc.vector.tensor_reduce(op=ALU.max, out=rminn[:], in_=f2neg[:],
                                    axis=AX.X)
            hi = sm.tile([128, 1], F32)
            neglo = sm.tile([128, 1], F32)
            nc.gpsimd.partition_all_reduce(hi[:], rmax[:], 128,
                                           bass_isa.ReduceOp.max)
            nc.gpsimd.partition_all_reduce(neglo[:], rminn[:], 128,
                                           bass_isa.ReduceOp.max)
            rng = sm.tile([128, 1], F32)
            inv = sm.tile([128, 1], F32)
            scl = sm.tile([128, 1], F32)
            nscl = sm.tile([128, 1], F32)
            sh = sm.tile([128, 1], F32)
            nc.vector.tensor_add(rng[:], hi[:], neglo[:])
            nc.vector.reciprocal(inv[:], rng[:])
            nc.vector.tensor_scalar(scl[:], inv[:], KEFF, None, ALU.mult)
            nc.vector.tensor_scalar(nscl[:], inv[:], -KEFF, None, ALU.mult)
            nc.vector.tensor_scalar(bc[:], f2v, neglo[:], scl[:],
                                    ALU.add, ALU.mult)
            nc.vector.tensor_sub(sh[:], ccol(C_B12), neglo[:])
            nc.vector.tensor_scalar(tcn[:], f1v, sh[:], nscl[:],
                                    ALU.add, ALU.mult)
            nc.vector.tensor_scalar(tcn[:], tcn[:], -1.0, float(KC - 4),
                                    ALU.max, ALU.min)
            nc.scalar.activation(e2sb[:], f2v, AF.Exp, bias=ccol(C_B2),
                                 scale=1.0)
            nc.scalar.activation(e2ssb[:], f2v, AF.Exp, bias=ccol(C_B2S),
                                 scale=0.01)
            nc.scalar.activation(e1[:], f1v, AF.Exp, bias=ccol(C_B1),
                                 scale=1.0)
            nc.scalar.activation(e1s[:], f1v, AF.Exp, bias=ccol(C_B1S),
                                 scale=0.01)
            nc.vector.tensor_scalar(e1sn[:], e1s[:], -1.0, None, ALU.mult)
            nc.vector.tensor_copy(e1s16[:], e1s[:])

            # bounce {tcn, e1, -e1s} and fp16 e1s through DRAM, read back
            # as partition-broadcasts / a partition-0 row
            for i, srcv in enumerate((tcn, e1, e1sn)):
                nc.sync.dma_start(
                    bounce_d[i, :].rearrange("(c p) -> p c", p=128), srcv[:]
                )
            nc.sync.dma_start(
                bounce16_d[:].rearrange("(c p) -> p c", p=128), e1s16[:]
            )
            tb = bp.tile([128, N], F32, tag="tb")
            e1b = bp.tile([128, N], F32, tag="e1b")
            e1snb = bp.tile([128, N], F32, tag="e1snb")
            nc.sync.dma_start(
                tb[:], bounce_d[0, :].rearrange("(o n) -> o n", o=1)
                .broadcast_to([128, N])
            )
            nc.sync.dma_start(
                e1b[:], bounce_d[1, :].rearrange("(o n) -> o n", o=1)
                .broadcast_to([128, N])
            )
            nc.sync.dma_start(
                e1snb[:], bounce_d[2, :].rearrange("(o n) -> o n", o=1)
                .broadcast_to([128, N])
            )
            nc.sync.dma_start(
                e1srow[:], bounce16_d[:].rearrange("(o n) -> o n", o=1)
            )

            # ---------------- scatter staircase weights ----------------
            ramp = qp.tile([128, N], F32, tag="qa")
            gs = qp.tile([128, N], F32, tag="qb")
            G1 = qp.tile([128, N], F16, tag="G1")
            G2 = qp.tile([128, N], F16, tag="G2")
            negk_b = negk.rearrange("p (o j) -> p o j", o=1) \
                         .broadcast_to([128, NCH, KC])
            bc_b = bc[:].rearrange("p (c o) -> p c o", o=1) \
                        .broadcast_to([128, NCH, KC])
            e2_b = e2sb[:].rearrange("p (c o) -> p c o", o=1) \
                          .broadcast_to([128, NCH, KC])
            e2s_b = e2ssb[:].rearrange("p (c o) -> p c o", o=1) \
                            .broadcast_to([128, NCH, KC])
            ramp3 = ramp[:].rearrange("p (c j) -> p c j", j=KC)
            gs3 = gs[:].rearrange("p (c j) -> p c j", j=KC)
            nc.vector.tensor_tensor(ramp3, negk_b, bc_b, ALU.add)
            nc.scalar.activation(gs[:], ramp[:], AF.Sigmoid, scale=SHARP)
            nc.vector.tensor_tensor(
                G1[:].rearrange("p (c j) -> p c j", j=KC), gs3, e2_b, ALU.mult
            )
            nc.vector.tensor_tensor(
                G2[:].rearrange("p (c j) -> p c j", j=KC), gs3, e2s_b, ALU.mult
            )

            # scatter matmuls: T[j, :] += G[:, c*128+j].T @ V16_c
            for c in range(NCH):
                nc.tensor.matmul(T1raw[:], G1[:, c * KC:(c + 1) * KC],
                                 V16[:, c * V16S:(c + 1) * V16S],
                                 start=(c == 0), stop=(c == NCH - 1))
                nc.tensor.matmul(T2raw[:], G2[:, c * KC:(c + 1) * KC],
                                 V16[:, c * V16S:(c + 1) * V16S],
                                 start=(c == 0), stop=(c == NCH - 1))

            # hat interpolation weights (overlap with scatter)
            qa = qp.tile([128, N], F32, tag="qa")
            qb = qp.tile([128, N], F32, tag="qb")
            q1 = qp.tile([128, N], F16, tag="q1")
            q2 = qp.tile([128, N], F16, tag="q2")
            nc.scalar.activation(qa[:], tb[:], AF.Relu, bias=ccol(C_HB1),
                                 scale=1.0)
            nc.scalar.activation(qb[:], tb[:], AF.Relu, bias=ccol(C_HB2),
                                 scale=-1.0)
            nc.vector.tensor_tensor(qa[:], qa[:], qb[:], ALU.min)
            nc.vector.tensor_mul(q1[:], qa[:], e1b[:])
            nc.vector.tensor_mul(q2[:], qa[:], e1snb[:])

            # tables to SBUF fp16
            nc.vector.tensor_copy(T1sb[:], T1raw[:])
            nc.scalar.copy(T2sb[:], T2raw[:])
            nc.vector.tensor_copy(t2row[:], T2raw[0:1, :])

            # ---------------- gather + epilogue ----------------
            with (
                tc.tile_pool(name="psG", bufs=4, space="PSUM") as psG,
                tc.tile_pool(name="outp", bufs=3) as op_,
                tc.tile_pool(name="rz", bufs=3) as rzp,
            ):
                for c in range(NCH):
                    gps = psG.tile([128, UC], F32, tag="gps")
                    nc.tensor.matmul(gps[:], q1[:, c * 128:(c + 1) * 128],
                                     T1sb[:], start=True, stop=False)
                    nc.tensor.matmul(gps[:], q2[:, c * 128:(c + 1) * 128],
                                     T2sb[:], start=False, stop=False)
                    nc.tensor.matmul(gps[:],
                                     e1srow[0:1, c * 128:(c + 1) * 128],
                                     t2row[:], start=False, stop=True)
                    rz = rzp.tile([128, 1], F32)
                    nc.vector.reciprocal(rz[:], gps[:, 0:1])
                    ob = op_.tile([128, D], F32)
                    nc.scalar.activation(ob[:], gps[:, 1:UC], AF.Relu,
                                         bias=ccol(C_BIAS), scale=rz[:])
                    nc.sync.dma_start(out_d[c * 128:(c + 1) * 128, :], ob[:])


def _build_nc():
    nc = bacc.Bacc("TRN2", target_bir_lowering=False, debug=False)
    seq_d = nc.dram_tensor("seq", [N, D], F32, kind="ExternalInput").ap()
    ww_d = nc.dram_tensor("ww", [D, DC], F32, kind="ExternalInput").ap()
    consts_d = nc.dram_tensor("consts", [128, 266], F32, kind="ExternalInput").ap()
    out_d = nc.dram_tensor("out", [N, D], F32, kind="ExternalOutput").ap()
    with tile.TileContext(nc) as tc:
        _emit(tc, seq_d, ww_d, consts_d, out_d)
    nc.compile()
    return nc


def _consts(b1, b2, bias):
    c = np.zeros((128, 266), dtype=np.float32)
    j = np.arange(KC, dtype=np.float32)
    c[:, C_NEGK:C_NEGK + KC] = (2.0 - j)[None, :]
    c[:, C_IDN:C_IDN + 128] = np.eye(128, dtype=np.float32)
    c[:, C_B1] = b1
    c[:, C_B2] = b2
    c[:, C_BIAS] = bias
    c[:, C_B1S] = 0.01 * b1
    c[:, C_B2S] = 0.01 * b2
    c[:, C_B12] = b1 + b2
    p = np.arange(128, dtype=np.float32)
    c[:, C_HB1] = 3.0 - p
    c[:, C_HB2] = p - 1.0
    return c


def _run(seq, W_fts, w_f1, b_f1, w_f2, b_f2, bias, trace=False):
    B = seq.shape[0]
    assert seq.shape == (B, N, D)
    ww = np.concatenate(
        [W_fts.astype(np.float32),
         w_f1.astype(np.float32).reshape(D, 1),
         w_f2.astype(np.float32).reshape(D, 1)], axis=1
    )
    consts = _consts(float(np.asarray(b_f1).ravel()[0]),
                     float(np.asarray(b_f2).ravel()[0]),
                     float(np.asarray(bias).ravel()[0]))
    nc = _build_nc()
    in_maps = [
        {"seq": np.ascontiguousarray(seq[b], dtype=np.float32),
         "ww": ww, "consts": consts}
        for b in range(B)
    ]
    res = run_bass_kernel_spmd(nc, in_maps, list(range(B)), trace=trace)
    out = np.stack([res.results[b]["out"] for b in range(B)]).astype(np.float32)
    return out, res


def kernel(seq, W_fts, w_f1, b_f1, w_f2, b_f2, bias):
    out, _ = _run(seq, W_fts, w_f1, b_f1, w_f2, b_f2, bias, trace=False)
    return out


# revision 22
# speedup vs baseline: 1.2031x; 1.2031x over previous
"""Trainium2 Bass kernel for nn_AttnHead (GAT-style attention head).

Reference computation per batch b:
    V   = seq @ W_fts                         [N, D]
    f1  = seq @ w_f1 + b_f1                   [N]
    f2  = seq @ w_f2 + b_f2                   [N]
    out = relu(softmax_m(lrelu(f1[n]+f2[m])) @ V + bias)

The logits are rank-1 (f1[n] + f2[m]) and exp(lrelu(x)) factorizes on each
side of x=0:
    exp(lrelu(f1+f2)) = e1[n]*e2[m]     if f1+f2 >= 0   (e = exp(f))
                      = e1s[n]*e2s[m]   otherwise       (es = exp(0.01 f))
so row n of the attention numerator is
    e1[n]*A(t_n) + e1s[n]*B(t_n),   t_n = -f1[n]
    A(t) = sum_{m: f2[m]>=t} e2[m] V[m,:]      (suffix sum over f2 order)
    B(t) = sum_{m: f2[m]<t}  e2s[m] V[m,:]     (prefix sum)
We evaluate A and the complementary suffix S2 on a uniform 128-bucket grid
over [min f2, max f2] via a smooth-staircase matmul (O(N*K*(2D+2)) instead
of O(N^2*D)), then linearly interpolate at each t_n with hat weights:
    out_unnorm[n] = sum_k hat_k(t_n) (e1[n] T1[k] - e1s[n] T2[k])
                    + e1s[n] * tot2          (since sum_k hat_k = 1)
where T1[k] = sum_m S(b_m-k) e2[m] [1|V], T2[k] = sum_m S(b_m-k) e2s[m[1|V],
tot2 = T2[k=-2] (the all-ones staircase column). All three terms accumulate
in one PSUM group (hat*e1, -hat*e1s, and a K=1 rank-1 matmul e1s x tot2).
Boundary smoothing is invisible at output precision because exp(lrelu) is
continuous at 0 and softmax normalization cancels smooth weight bias
(validated: absmax error ~9e-4 of output scale vs the fp32 reference).

Sharding: pure data-parallel, one batch per NeuronCore (B=8, 8 cores).
"""

import numpy as np

import concourse.bacc as bacc
import concourse.mybir as mybir
import concourse.tile as tile
from concourse import bass_isa
from concourse import library_config
from concourse.bass_utils import run_bass_kernel_spmd

F32 = mybir.dt.float32
F16 = mybir.dt.float16
AF = mybir.ActivationFunctionType
ALU = mybir.AluOpType
AX = mybir.AxisListType

N, D = 4096, 256
NCH = N // 128            # 32 m/n chunks of 128
KC = 128                  # staircase columns (grid buckets)
KEFF = float(KC - 7)      # usable buckets: b in [2, KC-5]
SHARP = 6.0               # sigmoid sharpness (in bucket units)
DC = D + 2                # phase-A matmul width: V | f1 | f2
UC = D + 1                # table width: weight col | weighted V
VS = D + 2                # Vsb chunk stride: V | f1 | f2
V16S = D + 1              # V16 chunk stride: 1 | V

# consts layout ([128, 264] fp32)
C_NEGK = 0       # cols 0:128   row p: 2.0 - j   (staircase ramp offsets)
C_IDN = 128      # cols 128:256 identity matrix (PE transpose operand)
C_B1 = 257       # col: b_f1
C_B2 = 258       # col: b_f2
C_BIAS = 259     # col: bias
C_B1S = 260      # col: 0.01*b_f1
C_B2S = 261      # col: 0.01*b_f2
C_B12 = 262      # col: b_f1 + b_f2
C_TWO = 264      # col: 2.0
C_NEGK3 = 266    # cols 266:394 row p: 3.0 - j   (hat left-arm offsets)
CW = 394         # consts width


def _emit(tc, seq_d, ww_d, consts_d, out_d):
    nc = tc.nc
    bounce16_d = nc.dram_tensor("bounce16", [N], F16).ap()
    nc.gpsimd.load_library(library_config.attn)

    with (
        tc.tile_pool(name="const", bufs=1) as cpool,
        tc.tile_pool(name="small", bufs=1) as sm,
    ):
        consts = cpool.tile([128, CW], F32)
        nc.sync.dma_start(consts[:], consts_d[:])
        ww32 = cpool.tile([128, 2 * DC], F32)
        for dc in range(2):
            nc.sync.dma_start(
                ww32[:, dc * DC:(dc + 1) * DC], ww_d[dc * 128:(dc + 1) * 128, :]
            )
        ww = cpool.tile([128, 2 * DC], F16)
        nc.vector.tensor_copy(ww[:], ww32[:])
        iden16 = cpool.tile([128, 128], F16)
        nc.vector.tensor_copy(iden16[:], consts[:, C_IDN:C_IDN + 128])
        negk = consts[:, C_NEGK:C_NEGK + KC]
        iden = consts[:, C_IDN:C_IDN + 128]

        def ccol(c):
            return consts[:, c:c + 1]

        e1 = sm.tile([128, NCH], F32)
        e1s = sm.tile([128, NCH], F32)
        e1sn = sm.tile([128, NCH], F32)        # -e1s
        e1s16 = sm.tile([128, NCH], F16)
        bc = sm.tile([128, NCH], F32)          # bucket coords of f2 (no +2)
        tcn = sm.tile([128, NCH], F32)         # clamped thresholds (no +2)
        e2sb = sm.tile([128, NCH], F32)        # exp(f2+b2)
        e2ssb = sm.tile([128, NCH], F32)       # exp(0.01(f2+b2))
        T1sb = sm.tile([128, UC], F16)
        T2sb = sm.tile([128, UC], F16)
        t2row = sm.tile([1, UC], F16)          # totals row (col k=-2)
        e1srow = sm.tile([1, N], F16)

        with (
            tc.tile_pool(name="psT", bufs=1, space="PSUM") as psT,
            tc.tile_pool(name="V", bufs=1) as Vp,
            tc.tile_pool(name="q", bufs=1) as qp,
        ):
            T1raw = psT.tile([128, UC], F32, tag="t1")
            T2raw = psT.tile([128, UC], F32, tag="t2")
            Vsb = Vp.tile([128, NCH * VS], F32)     # [V | f1 | f2] per chunk
            V16 = Vp.tile([128, NCH * V16S], F16)   # [1 | V] per chunk
            f1v = Vsb[:].rearrange("p (c s) -> p s c", s=VS)[:, D, :]
            f2v = Vsb[:].rearrange("p (c s) -> p s c", s=VS)[:, D + 1, :]

            # ---------------- phase A: seqT then V,f1,f2 ----------------
            with (
                tc.tile_pool(name="seqraw", bufs=6) as rawp,
                tc.tile_pool(name="seqT", bufs=1) as seqTp,
            ):
                seqT = seqTp.tile([128, 2 * N], F16)   # [d%128, dc*N + m]
                with tc.tile_pool(name="psA1", bufs=6, space="PSUM") as psA1:
                    for c in range(NCH):
                        raw = rawp.tile([128, D], F32)
                        nc.sync.dma_start(raw[:],
                                          seq_d[c * 128:(c + 1) * 128, :])
                        ptr = psA1.tile([128, 256], F32, tag="ptr")
                        for dc in range(2):
                            nc.tensor.transpose(
                                ptr[:, dc * 128:(dc + 1) * 128],
                                raw[:, dc * 128:(dc + 1) * 128],
                                iden,
                            )
                        dst = seqT[:].rearrange("p (dc m) -> p dc m", dc=2)[
                            :, :, c * 128:(c + 1) * 128
                        ]
                        src = ptr[:].rearrange("p (dc m) -> p dc m", dc=2)
                        if c % 2 == 0:
                            nc.vector.tensor_copy(dst, src)
                        else:
                            nc.scalar.copy(dst, src)

                with tc.tile_pool(name="psA2", bufs=4, space="PSUM") as psA2:
                    for c in range(NCH):
                        pa = psA2.tile([128, DC], F32, tag="pa")
                        for dc in range(2):
                            nc.tensor.matmul(
                                pa[:],
                                seqT[:, dc * N + c * 128:
                                     dc * N + (c + 1) * 128],
                                ww[:, dc * DC:(dc + 1) * DC],
                                start=(dc == 0),
                                stop=(dc == 1),
                            )
                        if c % 2 == 0:
                            nc.vector.tensor_copy(
                                Vsb[:, c * VS:(c + 1) * VS], pa[:]
                            )
                        else:
                            nc.scalar.copy(Vsb[:, c * VS:(c + 1) * VS], pa[:])

            # V16 = [1 | V] fp16: strided ones col + two half casts
            v16ones = V16[:].rearrange("p (c s) -> p s c", s=V16S)[:, 0:1, :]
            nc.vector.memset(v16ones, 1.0)
            half = NCH // 2
            src_v = Vsb[:].rearrange("p (c s) -> p c s", s=VS)[:, :, 0:D]
            dst_v = V16[:].rearrange("p (c s) -> p c s", s=V16S)[:, :, 1:]
            nc.vector.tensor_copy(dst_v[:, 0:half, :], src_v[:, 0:half, :])
            nc.scalar.copy(dst_v[:, half:, :], src_v[:, half:, :])

            # ---------------- grid + per-node scalars ----------------
            rmax = sm.tile([128, 1], F32)
            rminn = sm.tile([128, 1], F32)
            f2neg = sm.tile([128, NCH], F32)
            nc.vector.tensor_reduce(op=ALU.max, out=rmax[:], in_=f2v, axis=AX.X)
            nc.vector.tensor_scalar(f2neg[:], f2v, -1.0, None, ALU.mult)
            nc.vector.tensor_reduce(op=ALU.max, out=rminn[:], in_=f2neg[:],
                                    axis=AX.X)
            hi = sm.tile([128, 1], F32)
            neglo = sm.tile([128, 1], F32)
            nc.gpsimd.partition_all_reduce(hi[:], rmax[:], 128,
                                           bass_isa.ReduceOp.max)
            nc.gpsimd.partition_all_reduce(neglo[:], rminn[:], 128,
                                           bass_isa.ReduceOp.max)
            rng = sm.tile([128, 1], F32)
            inv = sm.tile([128, 1], F32)
            scl = sm.tile([128, 1], F32)
            nscl = sm.tile([128, 1], F32)
            sh = sm.tile([128, 1], F32)
            nc.vector.tensor_add(rng[:], hi[:], neglo[:])
            nc.vector.reciprocal(inv[:], rng[:])
            nc.vector.tensor_scalar(scl[:], inv[:], KEFF, None, ALU.mult)
            nc.vector.tensor_scalar(nscl[:], inv[:], -KEFF, None, ALU.mult)
            nc.vector.tensor_scalar(bc[:], f2v, neglo[:], scl[:],
                                    ALU.add, ALU.mult)
            nc.vector.tensor_sub(sh[:], ccol(C_B12), neglo[:])
            nc.vector.tensor_scalar(tcn[:], f1v, sh[:], nscl[:],
                                    ALU.add, ALU.mult)
            nc.vector.tensor_scalar(tcn[:], tcn[:], -1.0, float(KC - 4),
                                    ALU.max, ALU.min)
            nc.scalar.activation(e2sb[:], f2v, AF.Exp, bias=ccol(C_B2),
                                 scale=1.0)
            nc.scalar.activation(e2ssb[:], f2v, AF.Exp, bias=ccol(C_B2S),
                                 scale=0.01)
            nc.scalar.activation(e1[:], f1v, AF.Exp, bias=ccol(C_B1),
                                 scale=1.0)
            nc.scalar.activation(e1s[:], f1v, AF.Exp, bias=ccol(C_B1S),
                                 scale=0.01)
            nc.vector.tensor_scalar(e1sn[:], e1s[:], -1.0, None, ALU.mult)
            nc.vector.tensor_copy(e1s16[:], e1s[:])

            # e1s row for the rank-1 totals matmul: bounce through DRAM
            nc.sync.dma_start(
                bounce16_d[:].rearrange("(c p) -> p c", p=128), e1s16[:]
            )
            nc.sync.dma_start(
                e1srow[:], bounce16_d[:].rearrange("(o n) -> o n", o=1)
            )

            # ---------------- scatter staircase weights ----------------
            ramp = qp.tile([128, N], F32, tag="a1r")
            gs = qp.tile([128, N], F32, tag="a1")
            G1 = qp.tile([128, N], F16, tag="G1")
            G2 = qp.tile([128, N], F16, tag="G2")
            negk_b = negk.rearrange("p (o j) -> p o j", o=1) \
                         .broadcast_to([128, NCH, KC])
            bc_b = bc[:].rearrange("p (c o) -> p c o", o=1) \
                        .broadcast_to([128, NCH, KC])
            e2_b = e2sb[:].rearrange("p (c o) -> p c o", o=1) \
                          .broadcast_to([128, NCH, KC])
            e2s_b = e2ssb[:].rearrange("p (c o) -> p c o", o=1) \
                            .broadcast_to([128, NCH, KC])
            ramp3 = ramp[:].rearrange("p (c j) -> p c j", j=KC)
            gs3 = gs[:].rearrange("p (c j) -> p c j", j=KC)
            nc.vector.tensor_tensor(ramp3, negk_b, bc_b, ALU.add)
            nc.scalar.activation(gs[:], ramp[:], AF.Sigmoid, scale=SHARP)
            nc.vector.tensor_tensor(
                G1[:].rearrange("p (c j) -> p c j", j=KC), gs3, e2_b, ALU.mult
            )
            nc.vector.tensor_tensor(
                G2[:].rearrange("p (c j) -> p c j", j=KC), gs3, e2s_b, ALU.mult
            )

            # scatter matmuls: T[j, :] += G[:, c*128+j].T @ V16_c
            for c in range(NCH):
                nc.tensor.matmul(T1raw[:], G1[:, c * KC:(c + 1) * KC],
                                 V16[:, c * V16S:(c + 1) * V16S],
                                 start=(c == 0), stop=(c == NCH - 1))
                nc.tensor.matmul(T2raw[:], G2[:, c * KC:(c + 1) * KC],
                                 V16[:, c * V16S:(c + 1) * V16S],
                                 start=(c == 0), stop=(c == NCH - 1))

            # hat interpolation weights, built [n, k] (t/e1 per-partition)
            # then PE-transposed to the [k, n] layout the gather needs
            a1r = qp.tile([128, N], F32, tag="a1r")
            a1 = qp.tile([128, N], F32, tag="a1")
            a2 = qp.tile([128, N], F32, tag="a2")
            q1t = qp.tile([128, N], F16, tag="q1t")
            q2t = qp.tile([128, N], F16, tag="q2t")
            q1 = qp.tile([128, N], F16, tag="q1")
            q2 = qp.tile([128, N], F16, tag="q2")
            negk3 = consts[:, C_NEGK3:C_NEGK3 + KC]
            negk3_b = negk3.rearrange("p (o j) -> p o j", o=1) \
                           .broadcast_to([128, NCH, KC])
            tcn_b = tcn[:].rearrange("p (c o) -> p c o", o=1) \
                          .broadcast_to([128, NCH, KC])
            e1_b = e1[:].rearrange("p (c o) -> p c o", o=1) \
                        .broadcast_to([128, NCH, KC])
            e1sn_b = e1sn[:].rearrange("p (c o) -> p c o", o=1) \
                            .broadcast_to([128, NCH, KC])
            a1r3 = a1r[:].rearrange("p (c j) -> p c j", j=KC)
            # a1r = t - j + 3 ; left arm = relu(a1r), right arm = relu(2-a1r)
            nc.vector.tensor_tensor(a1r3, negk3_b, tcn_b, ALU.add)
            nc.scalar.activation(a1[:], a1r[:], AF.Relu, scale=1.0)
            nc.scalar.activation(a2[:], a1r[:], AF.Relu, bias=ccol(C_TWO),
                                 scale=-1.0)
            nc.vector.tensor_tensor(a1[:], a1[:], a2[:], ALU.min)
            nc.vector.tensor_tensor(
                q1t[:].rearrange("p (c j) -> p c j", j=KC),
                a1[:].rearrange("p (c j) -> p c j", j=KC), e1_b, ALU.mult)
            nc.vector.tensor_tensor(
                q2t[:].rearrange("p (c j) -> p c j", j=KC),
                a1[:].rearrange("p (c j) -> p c j", j=KC), e1sn_b, ALU.mult)
            with tc.tile_pool(name="psQ", bufs=2, space="PSUM") as psQ:
                for g in range(NCH // 4):
                    pq = psQ.tile([128, 4 * 128], F16, tag="pq")
                    for i in range(4):
                        c = g * 4 + i
                        nc.tensor.transpose(
                            pq[:, i * 128:(i + 1) * 128],
                            q1t[:, c * 128:(c + 1) * 128], iden16)
                    if g % 2 == 0:
                        nc.vector.tensor_copy(
                            q1[:, g * 512:(g + 1) * 512], pq[:])
                    else:
                        nc.scalar.copy(q1[:, g * 512:(g + 1) * 512], pq[:])
                for g in range(NCH // 4):
                    pq = psQ.tile([128, 4 * 128], F16, tag="pq")
                    for i in range(4):
                        c = g * 4 + i
                        nc.tensor.transpose(
                            pq[:, i * 128:(i + 1) * 128],
                            q2t[:, c * 128:(c + 1) * 128], iden16)
                    if g % 2 == 0:
                        nc.vector.tensor_copy(
                            q2[:, g * 512:(g + 1) * 512], pq[:])
                    else:
                        nc.scalar.copy(q2[:, g * 512:(g + 1) * 512], pq[:])

            # tables to SBUF fp16
            nc.vector.tensor_copy(T1sb[:], T1raw[:])
            nc.scalar.copy(T2sb[:], T2raw[:])
            nc.vector.tensor_copy(t2row[:], T2raw[0:1, :])

            # ---------------- gather + epilogue ----------------
            with (
                tc.tile_pool(name="psG", bufs=4, space="PSUM") as psG,
                tc.tile_pool(name="outp", bufs=3) as op_,
                tc.tile_pool(name="rz", bufs=3) as rzp,
            ):
                for c in range(NCH):
                    gps = psG.tile([128, UC], F32, tag="gps")
                    nc.tensor.matmul(gps[:], q1[:, c * 128:(c + 1) * 128],
                                     T1sb[:], start=True, stop=False)
                    nc.tensor.matmul(gps[:], q2[:, c * 128:(c + 1) * 128],
                                     T2sb[:], start=False, stop=False)
                    nc.tensor.matmul(gps[:],
                                     e1srow[0:1, c * 128:(c + 1) * 128],
                                     t2row[:], start=False, stop=True)
                    rz = rzp.tile([128, 1], F32)
                    nc.vector.reciprocal(rz[:], gps[:, 0:1])
                    ob = op_.tile([128, D], F32)
                    nc.scalar.activation(ob[:], gps[:, 1:UC], AF.Relu,
                                         bias=ccol(C_BIAS), scale=rz[:])
                    nc.sync.dma_start(out_d[c * 128:(c + 1) * 128, :], ob[:])


def _build_nc():
    nc = bacc.Bacc("TRN2", target_bir_lowering=False, debug=False)
    seq_d = nc.dram_tensor("seq", [N, D], F32, kind="ExternalInput").ap()
    ww_d = nc.dram_tensor("ww", [D, DC], F32, kind="ExternalInput").ap()
    consts_d = nc.dram_tensor("consts", [128, CW], F32, kind="ExternalInput").ap()
    out_d = nc.dram_tensor("out", [N, D], F32, kind="ExternalOutput").ap()
    with tile.TileContext(nc) as tc:
        _emit(tc, seq_d, ww_d, consts_d, out_d)
    nc.compile()
    return nc


def _consts(b1, b2, bias):
    c = np.zeros((128, CW), dtype=np.float32)
    j = np.arange(KC, dtype=np.float32)
    c[:, C_NEGK:C_NEGK + KC] = (2.0 - j)[None, :]
    c[:, C_IDN:C_IDN + 128] = np.eye(128, dtype=np.float32)
    c[:, C_B1] = b1
    c[:, C_B2] = b2
    c[:, C_BIAS] = bias
    c[:, C_B1S] = 0.01 * b1
    c[:, C_B2S] = 0.01 * b2
    c[:, C_B12] = b1 + b2
    c[:, C_TWO] = 2.0
    c[:, C_NEGK3:C_NEGK3 + KC] = (3.0 - j)[None, :]
    return c


def _run(seq, W_fts, w_f1, b_f1, w_f2, b_f2, bias, trace=False):
    B = seq.shape[0]
    assert seq.shape == (B, N, D)
    ww = np.concatenate(
        [W_fts.astype(np.float32),
         w_f1.astype(np.float32).reshape(D, 1),
         w_f2.astype(np.float32).reshape(D, 1)], axis=1
    )
    consts = _consts(float(np.asarray(b_f1).ravel()[0]),
                     float(np.asarray(b_f2).ravel()[0]),
                     float(np.asarray(bias).ravel()[0]))
    nc = _build_nc()
    in_maps = [
        {"seq": np.ascontiguousarray(seq[b], dtype=np.float32),
         "ww": ww, "consts": consts}
        for b in range(B)
    ]
    res = run_bass_kernel_spmd(nc, in_maps, list(range(B)), trace=trace)
    out = np.stack([res.results[b]["out"] for b in range(B)]).astype(np.float32)
    return out, res


def kernel(seq, W_fts, w_f1, b_f1, w_f2, b_f2, bias):
    out, _ = _run(seq, W_fts, w_f1, b_f1, w_f2, b_f2, bias, trace=False)
    return out


# revision 23
# speedup vs baseline: 1.2145x; 1.0094x over previous
"""Trainium2 Bass kernel for nn_AttnHead (GAT-style attention head).

Reference computation per batch b:
    V   = seq @ W_fts                         [N, D]
    f1  = seq @ w_f1 + b_f1                   [N]
    f2  = seq @ w_f2 + b_f2                   [N]
    out = relu(softmax_m(lrelu(f1[n]+f2[m])) @ V + bias)

The logits are rank-1 (f1[n] + f2[m]) and exp(lrelu(x)) factorizes on each
side of x=0:
    exp(lrelu(f1+f2)) = e1[n]*e2[m]     if f1+f2 >= 0   (e = exp(f))
                      = e1s[n]*e2s[m]   otherwise       (es = exp(0.01 f))
so row n of the attention numerator is
    e1[n]*A(t_n) + e1s[n]*B(t_n),   t_n = -f1[n]
    A(t) = sum_{m: f2[m]>=t} e2[m] V[m,:]      (suffix sum over f2 order)
    B(t) = sum_{m: f2[m]<t}  e2s[m] V[m,:]     (prefix sum)
We evaluate A and the complementary suffix S2 on a uniform 128-bucket grid
over [min f2, max f2] via a smooth-staircase matmul (O(N*K*(2D+2)) instead
of O(N^2*D)), then linearly interpolate at each t_n with hat weights:
    out_unnorm[n] = sum_k hat_k(t_n) (e1[n] T1[k] - e1s[n] T2[k])
                    + e1s[n] * tot2          (since sum_k hat_k = 1)
where T1[k] = sum_m S(b_m-k) e2[m] [1|V], T2[k] = sum_m S(b_m-k) e2s[m[1|V],
tot2 = T2[k=-2] (the all-ones staircase column). All three terms accumulate
in one PSUM group (hat*e1, -hat*e1s, and a K=1 rank-1 matmul e1s x tot2).
Boundary smoothing is invisible at output precision because exp(lrelu) is
continuous at 0 and softmax normalization cancels smooth weight bias
(validated: absmax error ~9e-4 of output scale vs the fp32 reference).

Sharding: pure data-parallel, one batch per NeuronCore (B=8, 8 cores).
"""

import numpy as np

import concourse.bacc as bacc
import concourse.mybir as mybir
import concourse.tile as tile
from concourse import bass_isa
from concourse import library_config
from concourse.bass_utils import run_bass_kernel_spmd

F32 = mybir.dt.float32
F16 = mybir.dt.float16
AF = mybir.ActivationFunctionType
ALU = mybir.AluOpType
AX = mybir.AxisListType

N, D = 4096, 256
NCH = N // 128            # 32 m/n chunks of 128
KC = 128                  # staircase columns (grid buckets)
KEFF = float(KC - 7)      # usable buckets: b in [2, KC-5]
SHARP = 6.0               # sigmoid sharpness (in bucket units)
DC = D + 2                # phase-A matmul width: V | f1 | f2
UC = D + 1                # table width: weight col | weighted V
VS = D + 2                # Vsb chunk stride: V | f1 | f2
V16S = D + 1              # V16 chunk stride: 1 | V

# consts layout ([128, 264] fp32)
C_NEGK = 0       # cols 0:128   row p: 2.0 - j   (staircase ramp offsets)
C_IDN = 128      # cols 128:256 identity matrix (PE transpose operand)
C_B1 = 257       # col: b_f1
C_B2 = 258       # col: b_f2
C_BIAS = 259     # col: bias
C_B1S = 260      # col: 0.01*b_f1
C_B2S = 261      # col: 0.01*b_f2
C_B12 = 262      # col: b_f1 + b_f2
C_TWO = 264      # col: 2.0
C_NEGK3 = 266    # cols 266:394 row p: 3.0 - j   (hat left-arm offsets)
CW = 394         # consts width


def _emit(tc, seq_d, ww_d, consts_d, out_d):
    nc = tc.nc
    nc.gpsimd.load_library(library_config.attn)

    with (
        tc.tile_pool(name="const", bufs=1) as cpool,
        tc.tile_pool(name="small", bufs=1) as sm,
    ):
        consts = cpool.tile([128, CW], F32)
        nc.sync.dma_start(consts[:], consts_d[:])
        ww32 = cpool.tile([128, 2 * DC], F32)
        for dc in range(2):
            nc.sync.dma_start(
                ww32[:, dc * DC:(dc + 1) * DC], ww_d[dc * 128:(dc + 1) * 128, :]
            )
        ww = cpool.tile([128, 2 * DC], F16)
        nc.vector.tensor_copy(ww[:], ww32[:])
        iden16 = cpool.tile([128, 128], F16)
        nc.vector.tensor_copy(iden16[:], consts[:, C_IDN:C_IDN + 128])
        negk = consts[:, C_NEGK:C_NEGK + KC]
        iden = consts[:, C_IDN:C_IDN + 128]

        def ccol(c):
            return consts[:, c:c + 1]

        e1 = sm.tile([128, NCH], F32)
        e1s = sm.tile([128, NCH], F32)
        bc = sm.tile([128, NCH], F32)          # bucket coords of f2 (no +2)
        tcn = sm.tile([128, NCH], F32)         # clamped thresholds (no +2)
        e2sb = sm.tile([128, NCH], F32)        # exp(f2+b2)
        e2ssb = sm.tile([128, NCH], F32)       # exp(0.01(f2+b2))
        T1sb = sm.tile([128, UC], F16)
        T2sb = sm.tile([128, UC], F16)
        tot2r = sm.tile([1, UC], F32)          # totals row (col k=-2)
        tot2b = sm.tile([128, UC], F32)

        with (
            tc.tile_pool(name="psT", bufs=1, space="PSUM") as psT,
            tc.tile_pool(name="V", bufs=1) as Vp,
            tc.tile_pool(name="q", bufs=1) as qp,
        ):
            T1raw = psT.tile([128, UC], F32, tag="t1")
            T2raw = psT.tile([128, UC], F32, tag="t2")
            Vsb = Vp.tile([128, NCH * VS], F32)     # [V | f1 | f2] per chunk
            V16 = Vp.tile([128, NCH * V16S], F16)   # [1 | V] per chunk
            f1v = Vsb[:].rearrange("p (c s) -> p s c", s=VS)[:, D, :]
            f2v = Vsb[:].rearrange("p (c s) -> p s c", s=VS)[:, D + 1, :]

            # ---------------- phase A: seqT then V,f1,f2 ----------------
            with (
                tc.tile_pool(name="seqraw", bufs=6) as rawp,
                tc.tile_pool(name="seqT", bufs=1) as seqTp,
            ):
                seqT = seqTp.tile([128, 2 * N], F16)   # [d%128, dc*N + m]
                with tc.tile_pool(name="psA1", bufs=6, space="PSUM") as psA1:
                    for c in range(NCH):
                        raw = rawp.tile([128, D], F32)
                        nc.sync.dma_start(raw[:],
                                          seq_d[c * 128:(c + 1) * 128, :])
                        ptr = psA1.tile([128, 256], F32, tag="ptr")
                        for dc in range(2):
                            nc.tensor.transpose(
                                ptr[:, dc * 128:(dc + 1) * 128],
                                raw[:, dc * 128:(dc + 1) * 128],
                                iden,
                            )
                        dst = seqT[:].rearrange("p (dc m) -> p dc m", dc=2)[
                            :, :, c * 128:(c + 1) * 128
                        ]
                        src = ptr[:].rearrange("p (dc m) -> p dc m", dc=2)
                        if c % 2 == 0:
                            nc.vector.tensor_copy(dst, src)
                        else:
                            nc.scalar.copy(dst, src)

                with tc.tile_pool(name="psA2", bufs=4, space="PSUM") as psA2:
                    for c in range(NCH):
                        pa = psA2.tile([128, DC], F32, tag="pa")
                        for dc in range(2):
                            nc.tensor.matmul(
                                pa[:],
                                seqT[:, dc * N + c * 128:
                                     dc * N + (c + 1) * 128],
                                ww[:, dc * DC:(dc + 1) * DC],
                                start=(dc == 0),
                                stop=(dc == 1),
                            )
                        if c % 2 == 0:
                            nc.vector.tensor_copy(
                                Vsb[:, c * VS:(c + 1) * VS], pa[:]
                            )
                        else:
                            nc.scalar.copy(Vsb[:, c * VS:(c + 1) * VS], pa[:])

            # V16 = [1 | V] fp16: strided ones col + two half casts
            v16ones = V16[:].rearrange("p (c s) -> p s c", s=V16S)[:, 0:1, :]
            nc.vector.memset(v16ones, 1.0)
            half = NCH // 2
            src_v = Vsb[:].rearrange("p (c s) -> p c s", s=VS)[:, :, 0:D]
            dst_v = V16[:].rearrange("p (c s) -> p c s", s=V16S)[:, :, 1:]
            nc.vector.tensor_copy(dst_v[:, 0:half, :], src_v[:, 0:half, :])
            nc.scalar.copy(dst_v[:, half:, :], src_v[:, half:, :])

            # ---------------- grid + per-node scalars ----------------
            rmax = sm.tile([128, 1], F32)
            rminn = sm.tile([128, 1], F32)
            f2neg = sm.tile([128, NCH], F32)
            nc.vector.tensor_reduce(op=ALU.max, out=rmax[:], in_=f2v, axis=AX.X)
            nc.vector.tensor_scalar(f2neg[:], f2v, -1.0, None, ALU.mult)
            nc.vector.tensor_reduce(op=ALU.max, out=rminn[:], in_=f2neg[:],
                                    axis=AX.X)
            hi = sm.tile([128, 1], F32)
            neglo = sm.tile([128, 1], F32)
            nc.gpsimd.partition_all_reduce(hi[:], rmax[:], 128,
                                           bass_isa.ReduceOp.max)
            nc.gpsimd.partition_all_reduce(neglo[:], rminn[:], 128,
                                           bass_isa.ReduceOp.max)
            rng = sm.tile([128, 1], F32)
            inv = sm.tile([128, 1], F32)
            scl = sm.tile([128, 1], F32)
            nscl = sm.tile([128, 1], F32)
            sh = sm.tile([128, 1], F32)
            nc.vector.tensor_add(rng[:], hi[:], neglo[:])
            nc.vector.reciprocal(inv[:], rng[:])
            nc.vector.tensor_scalar(scl[:], inv[:], KEFF, None, ALU.mult)
            nc.vector.tensor_scalar(nscl[:], inv[:], -KEFF, None, ALU.mult)
            nc.vector.tensor_scalar(bc[:], f2v, neglo[:], scl[:],
                                    ALU.add, ALU.mult)
            nc.vector.tensor_sub(sh[:], ccol(C_B12), neglo[:])
            nc.vector.tensor_scalar(tcn[:], f1v, sh[:], nscl[:],
                                    ALU.add, ALU.mult)
            nc.vector.tensor_scalar(tcn[:], tcn[:], -1.0, float(KC - 4),
                                    ALU.max, ALU.min)
            nc.scalar.activation(e2sb[:], f2v, AF.Exp, bias=ccol(C_B2),
                                 scale=1.0)
            nc.scalar.activation(e2ssb[:], f2v, AF.Exp, bias=ccol(C_B2S),
                                 scale=0.01)
            nc.scalar.activation(e1[:], f1v, AF.Exp, bias=ccol(C_B1),
                                 scale=1.0)
            nc.scalar.activation(e1s[:], f1v, AF.Exp, bias=ccol(C_B1S),
                                 scale=0.01)

            # ---------------- scatter staircase weights ----------------
            ramp = qp.tile([128, N], F32, tag="a1r")
            gs = qp.tile([128, N], F32, tag="a1")
            G1 = qp.tile([128, N], F16, tag="G1")
            G2 = qp.tile([128, N], F16, tag="G2")
            negk_b = negk.rearrange("p (o j) -> p o j", o=1) \
                         .broadcast_to([128, NCH, KC])
            bc_b = bc[:].rearrange("p (c o) -> p c o", o=1) \
                        .broadcast_to([128, NCH, KC])
            e2_b = e2sb[:].rearrange("p (c o) -> p c o", o=1) \
                          .broadcast_to([128, NCH, KC])
            e2s_b = e2ssb[:].rearrange("p (c o) -> p c o", o=1) \
                            .broadcast_to([128, NCH, KC])
            ramp3 = ramp[:].rearrange("p (c j) -> p c j", j=KC)
            gs3 = gs[:].rearrange("p (c j) -> p c j", j=KC)
            nc.vector.tensor_tensor(ramp3, negk_b, bc_b, ALU.add)
            nc.scalar.activation(gs[:], ramp[:], AF.Sigmoid, scale=SHARP)
            nc.vector.tensor_tensor(
                G1[:].rearrange("p (c j) -> p c j", j=KC), gs3, e2_b, ALU.mult
            )
            nc.vector.tensor_tensor(
                G2[:].rearrange("p (c j) -> p c j", j=KC), gs3, e2s_b, ALU.mult
            )

            # scatter matmuls: T[j, :] += G[:, c*128+j].T @ V16_c
            for c in range(NCH):
                nc.tensor.matmul(T1raw[:], G1[:, c * KC:(c + 1) * KC],
                                 V16[:, c * V16S:(c + 1) * V16S],
                                 start=(c == 0), stop=(c == NCH - 1))
                nc.tensor.matmul(T2raw[:], G2[:, c * KC:(c + 1) * KC],
                                 V16[:, c * V16S:(c + 1) * V16S],
                                 start=(c == 0), stop=(c == NCH - 1))

            # hat interpolation weights, built [n, k] (t/e1 per-partition)
            # then PE-transposed to the [k, n] layout the gather needs
            a1r = qp.tile([128, N], F32, tag="a1r")
            a1 = qp.tile([128, N], F32, tag="a1")
            a2 = qp.tile([128, N], F32, tag="a2")
            q1t = qp.tile([128, N], F16, tag="q1t")
            q2t = qp.tile([128, N], F16, tag="q2t")
            q1 = qp.tile([128, N], F16, tag="q1")
            q2 = qp.tile([128, N], F16, tag="q2")
            negk3 = consts[:, C_NEGK3:C_NEGK3 + KC]
            negk3_b = negk3.rearrange("p (o j) -> p o j", o=1) \
                           .broadcast_to([128, NCH, KC])
            tcn_b = tcn[:].rearrange("p (c o) -> p c o", o=1) \
                          .broadcast_to([128, NCH, KC])
            e1_b = e1[:].rearrange("p (c o) -> p c o", o=1) \
                        .broadcast_to([128, NCH, KC])
            e1s_b = e1s[:].rearrange("p (c o) -> p c o", o=1) \
                          .broadcast_to([128, NCH, KC])
            a1r3 = a1r[:].rearrange("p (c j) -> p c j", j=KC)
            # a1r = t - j + 3 ; left arm = relu(a1r), right arm = relu(2-a1r)
            nc.vector.tensor_tensor(a1r3, negk3_b, tcn_b, ALU.add)
            nc.scalar.activation(a1[:], a1r[:], AF.Relu, scale=1.0)
            nc.scalar.activation(a2[:], a1r[:], AF.Relu, bias=ccol(C_TWO),
                                 scale=-1.0)
            nc.vector.tensor_tensor(a1[:], a1[:], a2[:], ALU.min)
            nc.vector.tensor_tensor(
                q1t[:].rearrange("p (c j) -> p c j", j=KC),
                a1[:].rearrange("p (c j) -> p c j", j=KC), e1_b, ALU.mult)
            nc.vector.tensor_tensor(
                q2t[:].rearrange("p (c j) -> p c j", j=KC),
                a1[:].rearrange("p (c j) -> p c j", j=KC), e1s_b, ALU.mult)
            with tc.tile_pool(name="psQ", bufs=2, space="PSUM") as psQ:
                for g in range(NCH // 4):
                    pq = psQ.tile([128, 4 * 128], F16, tag="pq")
                    for i in range(4):
                        c = g * 4 + i
                        nc.tensor.transpose(
                            pq[:, i * 128:(i + 1) * 128],
                            q1t[:, c * 128:(c + 1) * 128], iden16)
                    if g % 2 == 0:
                        nc.vector.tensor_copy(
                            q1[:, g * 512:(g + 1) * 512], pq[:])
                    else:
                        nc.scalar.copy(q1[:, g * 512:(g + 1) * 512], pq[:])
                for g in range(NCH // 4):
                    pq = psQ.tile([128, 4 * 128], F16, tag="pq")
                    for i in range(4):
                        c = g * 4 + i
                        nc.tensor.transpose(
                            pq[:, i * 128:(i + 1) * 128],
                            q2t[:, c * 128:(c + 1) * 128], iden16)
                    if g % 2 == 0:
                        nc.vector.tensor_copy(
                            q2[:, g * 512:(g + 1) * 512], pq[:])
                    else:
                        nc.scalar.copy(q2[:, g * 512:(g + 1) * 512], pq[:])

            # tables to SBUF fp16; T2 = tot2 - suffix (prefix table)
            nc.vector.tensor_copy(T1sb[:], T1raw[:])
            nc.vector.tensor_copy(tot2r[:], T2raw[0:1, :])
            nc.gpsimd.partition_broadcast(tot2b[:], tot2r[:], 128)
            nc.vector.tensor_tensor(T2sb[:], tot2b[:], T2raw[:],
                                    ALU.subtract)

            # ---------------- gather + epilogue ----------------
            with (
                tc.tile_pool(name="psG", bufs=4, space="PSUM") as psG,
                tc.tile_pool(name="outp", bufs=3) as op_,
                tc.tile_pool(name="rz", bufs=3) as rzp,
            ):
                for c in range(NCH):
                    gps = psG.tile([128, UC], F32, tag="gps")
                    nc.tensor.matmul(gps[:], q1[:, c * 128:(c + 1) * 128],
                                     T1sb[:], start=True, stop=False)
                    nc.tensor.matmul(gps[:], q2[:, c * 128:(c + 1) * 128],
                                     T2sb[:], start=False, stop=True)
                    rz = rzp.tile([128, 1], F32)
                    nc.vector.reciprocal(rz[:], gps[:, 0:1])
                    ob = op_.tile([128, D], F32)
                    nc.scalar.activation(ob[:], gps[:, 1:UC], AF.Relu,
                                         bias=ccol(C_BIAS), scale=rz[:])
                    nc.sync.dma_start(out_d[c * 128:(c + 1) * 128, :], ob[:])


def _build_nc():
    nc = bacc.Bacc("TRN2", target_bir_lowering=False, debug=False)
    seq_d = nc.dram_tensor("seq", [N, D], F32, kind="ExternalInput").ap()
    ww_d = nc.dram_tensor("ww", [D, DC], F32, kind="ExternalInput").ap()
    consts_d = nc.dram_tensor("consts", [128, CW], F32, kind="ExternalInput").ap()
    out_d = nc.dram_tensor("out", [N, D], F32, kind="ExternalOutput").ap()
    with tile.TileContext(nc) as tc:
        _emit(tc, seq_d, ww_d, consts_d, out_d)
    nc.compile()
    return nc


def _consts(b1, b2, bias):
    c = np.zeros((128, CW), dtype=np.float32)
    j = np.arange(KC, dtype=np.float32)
    c[:, C_NEGK:C_NEGK + KC] = (2.0 - j)[None, :]
    c[:, C_IDN:C_IDN + 128] = np.eye(128, dtype=np.float32)
    c[:, C_B1] = b1
    c[:, C_B2] = b2
    c[:, C_BIAS] = bias
    c[:, C_B1S] = 0.01 * b1
    c[:, C_B2S] = 0.01 * b2
    c[:, C_B12] = b1 + b2
    c[:, C_TWO] = 2.0
    c[:, C_NEGK3:C_NEGK3 + KC] = (3.0 - j)[None, :]
    return c


def _run(seq, W_fts, w_f1, b_f1, w_f2, b_f2, bias, trace=False):
    B = seq.shape[0]
    assert seq.shape == (B, N, D)
    ww = np.concatenate(
        [W_fts.astype(np.float32),
         w_f1.astype(np.float32).reshape(D, 1),
         w_f2.astype(np.float32).reshape(D, 1)], axis=1
    )
    consts = _consts(float(np.asarray(b_f1).ravel()[0]),
                     float(np.asarray(b_f2).ravel()[0]),
                     float(np.asarray(bias).ravel()[0]))
    nc = _build_nc()
    in_maps = [
        {"seq": np.ascontiguousarray(seq[b], dtype=np.float32),
         "ww": ww, "consts": consts}
        for b in range(B)
    ]
    res = run_bass_kernel_spmd(nc, in_maps, list(range(B)), trace=trace)
    out = np.stack([res.results[b]["out"] for b in range(B)]).astype(np.float32)
    return out, res


def kernel(seq, W_fts, w_f1, b_f1, w_f2, b_f2, bias):
    out, _ = _run(seq, W_fts, w_f1, b_f1, w_f2, b_f2, bias, trace=False)
    return out
